# revision 29
# baseline (speedup 1.0000x reference)
"""LIF (leaky integrate-and-fire) recurrence kernel for Trainium2, 8 NeuronCores.

Problem: x (T=32, B=64, N=32768) f32.
    m[t] = tau*v[t-1] + x[t];  y[t] = (m[t] >= 1.0);  v[t] = m[t]*(1-y[t])
Output: y (32, 64, 32768) f32.

Sharding: data-parallel over batch. Core c handles x[:, 8c:8(c+1), :],
a (32, 262144)-element independent recurrence.

Per-core pipeline (bit-exact vs the f32 reference):
  DVE — ONE custom-DVE op per timestep (registered at import, the
  supported dve_ops.OPS extension path). State is m (not v); the reset
  folds into the next step's read:
      m[t] = select(m[t-1] < vth, m[t-1], 0) * tau + x[t]
  4 ALU stages, 1 elem/cycle -> ~2.2us/step instead of the 2-op
  scalar_tensor_tensor chain (~4.6us/step).
  ACT — s = Sign(m - c) with c = 1 - 2^-24 (the f32 just below vth):
  s = +1  <=>  m >= vth exactly (Sterbenz); s in {-1,0,+1} written bf16.
  PE  — packs 8 signs into one f32: out[g,f] = sum_j 4^j * s[8g+j,f],
  a balanced base-4 integer in [-21845, 21845], exact in f32 (bf16
  weights 4^j are exact powers of two; every product and partial sum is
  an exact small integer). The host adds 21845 and reads 2-bit digits:
  y = (digit == 2). This packs y to 0.5 B/element, halving the y DMA
  traffic vs int8 and quartering it vs bf16.

x loads are staged [1,3,4,...,4,2,1,1] timesteps (fast fill + short
drain) on the sync HWDGE ring; the packed y (PSUM [16, 2048] f32) is
stored every timestep on the scalar ring.

The kernel is DMA-bound: 33.55MB x in + 4.19MB y out per core at
~25.4GB/s/engine x 16 engines -> ~93us of DMA engine time; DVE ~70us,
ACT ~61us, PE ~27us all hide under it.
"""

import sys

if "/opt/trn_rl_repo" not in sys.path:
    sys.path.insert(0, "/opt/trn_rl_repo")

import numpy as np

TAU = 0.5
V_TH = 1.0
C_TH = 1.0 - 2.0 ** -24      # largest f32 < V_TH

N_CORES = 8
T, B, N = 32, 64, 32768
B_SH = B // N_CORES          # 8 batch rows per core
E = B_SH * N                 # 262144 elements per core per timestep
P = 128                      # SBUF partitions
F = E // P                   # 2048 f32 per partition per timestep

PACK_BIAS = (4 ** 4 - 1) // 3  # 85: balanced -> unsigned base-4 digits
MM = 512                     # matmul moving free-dim limit; 4 bands per step

X_CHUNKS = [1, 3] + [4] * 6 + [2, 1, 1]  # timesteps per x load
Y_CHUNKS = [4] * 7 + [2, 1, 1]           # timesteps per packed-y store

_compiled = None


def _register_lif_op():
    """Register the fused LIF step as a custom DVE op (dve_ops.OPS append,
    the documented extension path; the uop table is generated per-NEFF)."""
    import concourse.dve_ops as dve_ops
    from concourse.dve_spec import (
        Spec, Src0, Src1, C0, C1, Zero, select, lower, _has_src1,
    )
    from concourse.dve_uop import DveOpSpec

    name = "LIF_STEP_ANT"
    for op in dve_ops.OPS:
        if op.name == name:
            return op
    body = select(Src0 < C1, Src0, Zero) * C0 + Src1
    spec = Spec(
        body=body,
        reference=lambda in0, in1, s0, s1, imm2: (
            np.where(in0 < s1, in0, np.float32(0.0)).astype(np.float32)
            * np.float32(s0) + in1
        ).astype(np.float32),
    )
    row = dve_ops._CUSTOM_DVE_ROW_BASE + len(dve_ops.OPS)
    assert row < 0x20
    dve_ops._SUB_OPCODE_FOR_NAME[name] = row
    sha = DveOpSpec(
        name=name, uops=lower(spec, ver="v3"), rd1_en=_has_src1(spec)
    ).sha("v3")
    op = dve_ops.DveOp(name, spec, subdim=False, uops_sha={"v3": sha})
    dve_ops.OPS.append(op)
    dve_ops.CUSTOM_DVE_SPECS[name] = spec
    return op


def _pack_weights() -> np.ndarray:
    """[128, 32] bf16: W[p, g] = 4^(p%4) for g == p//4 else 0 (exact in bf16)."""
    import ml_dtypes

    w = np.zeros((P, 32), dtype=np.float32)
    for p in range(P):
        w[p, p // 4] = float(4 ** (p % 4))
    return w.astype(ml_dtypes.bfloat16)


def _build():
    from concourse import bacc, tile, mybir
    from concourse.bass import MemorySpace

    lif_op = _register_lif_op()
    f32 = mybir.dt.float32
    bf16 = mybir.dt.bfloat16
    assert sum(X_CHUNKS) == T
    nc = bacc.Bacc("TRN2", debug=False, num_devices=N_CORES)
    x = nc.dram_tensor("x", [T, E], f32, kind="ExternalInput").ap()
    w = nc.dram_tensor("w", [P, 32], bf16, kind="ExternalInput").ap()
    yp = nc.dram_tensor("yp", [P, T, MM], bf16, kind="ExternalOutput").ap()

    x_r = x.rearrange("t (p f) -> t p f", p=P)  # [t, p, f] view of DRAM

    with tile.TileContext(nc) as tc:
        with (
            tc.tile_pool(name="io", bufs=3) as io_pool,
            tc.tile_pool(name="state", bufs=1) as st_pool,
            tc.tile_pool(name="m", bufs=6) as m_pool,
            tc.tile_pool(name="s", bufs=4) as s_pool,
            tc.tile_pool(name="ps", bufs=4, space=MemorySpace.PSUM) as ps_pool,
            tc.tile_pool(name="yb", bufs=3) as y_pool,
        ):
            cb = st_pool.tile([P, 1], f32, tag="cb")     # Sign bias = -C_TH
            nc.gpsimd.memset(cb[:], -C_TH)
            m_prev = st_pool.tile([P, F], f32, tag="m0")  # v[-1] = 0 seed
            nc.gpsimd.memset(m_prev[:], 0.0)
            wt = st_pool.tile([P, 32], bf16, tag="w")
            nc.sync.dma_start(out=wt[:], in_=w)

            # issue x loads lazily, two chunks ahead of consumption
            x_tiles = {}          # t -> (tile, col offset)
            next_chunk = 0
            t_loaded = 0

            def load_chunk():
                nonlocal next_chunk, t_loaded
                n_t = X_CHUNKS[next_chunk]
                xt = io_pool.tile([P, 4 * F], f32, tag="x")
                nc.sync.dma_start(
                    out=xt[:, : n_t * F].rearrange("p (t f) -> p t f", t=n_t),
                    in_=x_r[t_loaded:t_loaded + n_t].rearrange("t p f -> p t f"),
                )
                for i in range(n_t):
                    x_tiles[t_loaded + i] = (xt, i * F)
                next_chunk += 1
                t_loaded += n_t

            load_chunk()
            y_t = None
            y_chunk_idx = 0
            y_off = 0  # timesteps into current y chunk
            deferred = {}  # emit-step -> (psum tile, y slot) for DVE copies

            def flush_deferred(t_emit):
                # DVE PSUM->SBUF copy, deferred 2 steps so its matmul dep is
                # long done and the in-order DVE queue never stalls on it
                pk_d, yslot_d = deferred.pop(t_emit)
                nc.vector.tensor_scalar(
                    out=yslot_d, in0=pk_d[:], scalar1=1.0, scalar2=None,
                    op0=mybir.AluOpType.mult,
                )

            for t in range(T):
                if t not in x_tiles:
                    load_chunk()
                if next_chunk < len(X_CHUNKS) and t == t_loaded - X_CHUNKS[next_chunk - 1]:
                    load_chunk()  # prefetch one chunk ahead
                xt, off = x_tiles.pop(t)
                xs = xt[:, off:off + F]
                m = m_pool.tile([P, F], f32, tag="m")
                # m = select(m_prev < vth, m_prev, 0) * tau + x_t
                nc.vector._custom_dve(
                    lif_op, out=m[:], in0=m_prev[:], in1=xs, s0=TAU, s1=V_TH,
                )
                if t in deferred:
                    flush_deferred(t)
                # s = Sign(m - c) -> bf16 in {-1, 0, +1}; +1 iff m >= vth
                s = s_pool.tile([P, F], bf16, tag="s")
                nc.scalar.activation(
                    out=s[:], in_=m[:],
                    func=mybir.ActivationFunctionType.Sign,
                    bias=cb[:], scale=1.0,
                )
                m_prev = m
                # PE pack (4 signs -> one bf16-exact integer in [-85, 85]):
                # matmul q packs columns [512q, 512q+512) into the 32-row
                # band at PSUM base partition 32q — all 128 partitions hold
                # real data:  pk[32q+g, f'] = sum_{j<4} 4^j * s[4g+j, 512q+f']
                pk = ps_pool.tile([P, MM], f32, tag="pk")
                for q in range(4):
                    nc.tensor.matmul(
                        out=pk[32 * q:32 * (q + 1), :],
                        lhsT=wt[:],
                        rhs=s[:, MM * q:MM * (q + 1)],
                        start=True, stop=True,
                        tile_position=(0, 32 * q),
                    )
                # PSUM -> SBUF bounce (512 cycles) narrowing to bf16 (exact:
                # small integers). 2/3 on ACT immediately; 1/3 on DVE deferred
                # 2 steps, balancing ACT (~Sign+2/3 copy) against DVE.
                n_yt = Y_CHUNKS[y_chunk_idx]
                if y_off == 0:
                    y_t = y_pool.tile([P, max(Y_CHUNKS) * MM], bf16, tag="y")
                yslot = y_t[:, y_off * MM:(y_off + 1) * MM]
                if y_off < 2 and n_yt == 4:
                    # lands at y_off+2 in the same chunk, before its store
                    deferred[t + 2] = (pk, yslot)
                else:
                    nc.scalar.copy(out=yslot, in_=pk[:])
                y_off += 1
                if y_off == n_yt:
                    assert not deferred, deferred
                    nc.scalar.dma_start(
                        out=yp[:, t - n_yt + 1:t + 1, :],
                        in_=y_t[:, : n_yt * MM].rearrange("p (t f) -> p t f", t=n_yt),
                    )
                    y_chunk_idx += 1
                    y_off = 0
    nc.compile()
    return nc


def _get_compiled():
    global _compiled
    if _compiled is None:
        _compiled = _build()
        # warm the NEFF (first execution pays ~20us of cold-start)
        import concourse.bass_utils as bass_utils

        z = [
            {"x": np.zeros((T, E), dtype=np.float32), "w": _pack_weights()}
            for _ in range(N_CORES)
        ]
        bass_utils.run_bass_kernel_spmd(
            _compiled, z, core_ids=list(range(N_CORES))
        )
    return _compiled


def kernel(x: np.ndarray, _trace: bool = False):
    import concourse.bass_utils as bass_utils

    nc = _get_compiled()
    x = np.ascontiguousarray(x, dtype=np.float32)
    wv = _pack_weights()
    in_maps = [
        {"x": x[:, c * B_SH:(c + 1) * B_SH, :].reshape(T, E), "w": wv}
        for c in range(N_CORES)
    ]
    res = bass_utils.run_bass_kernel_spmd(
        nc, in_maps, core_ids=list(range(N_CORES)), trace=_trace
    )
    y = np.empty((T, B, N), dtype=np.float32)
    shifts = (2 * np.arange(4, dtype=np.int32))[None, None, :, None, None]
    for c in range(N_CORES):
        v32 = res.results[c]["yp"].astype(np.int32) + PACK_BIAS  # [128, T, 512]
        # row 32q + g, step t, word f', digit j -> element (4g+j, 512q+f')
        d = (v32.reshape(4, 32, 1, T, MM) >> shifts) & 3         # [q, g, j, t, f']
        yb = (d == 2).transpose(3, 1, 2, 0, 4)                   # [t, g, j, q, f']
        y[:, c * B_SH:(c + 1) * B_SH, :] = yb.reshape(T, B_SH, N)
    if _trace:
        return y, res
    return y


# revision 30
# speedup vs baseline: 1.1302x; 1.1302x over previous
"""LIF (leaky integrate-and-fire) recurrence kernel for Trainium2, 8 NeuronCores.

Problem: x (T=32, B=64, N=32768) f32.
    m[t] = tau*v[t-1] + x[t];  y[t] = (m[t] >= 1.0);  v[t] = m[t]*(1-y[t])
Output: y (32, 64, 32768) f32.

Sharding: data-parallel over batch. Core c handles x[:, 8c:8(c+1), :],
a (32, 262144)-element independent recurrence.

Per-core pipeline (bit-exact vs the f32 reference):
  DVE — ONE custom-DVE op per timestep (registered at import, the
  supported dve_ops.OPS extension path). State is m (not v); the reset
  folds into the next step's read:
      m[t] = select(m[t-1] < vth, m[t-1], 0) * tau + x[t]
  4 ALU stages, 1 elem/cycle -> ~2.2us/step instead of the 2-op
  scalar_tensor_tensor chain (~4.6us/step).
  ACT — s = Sign(m - c) with c = 1 - 2^-24 (the f32 just below vth):
  s = +1  <=>  m >= vth exactly (Sterbenz); s in {-1,0,+1} written bf16.
  PE  — packs 4 signs into one word: pk[32q+g, f'] = sum_{j<4} 4^j *
  s[4g+j, 512q+f'], a balanced base-4 integer in [-85, 85] — exact in
  bf16 (weights 4^j are exact powers of two; every product and partial
  sum is an exact small integer; matmul q fills the 32-partition band
  at PSUM base 32q, so all 128 partitions carry real data and the
  stores get full-rate 4KB descriptors). The host adds 85 and reads
  2-bit digits: y = (digit == 2). 0.5 B/element of y DMA traffic.
  The PSUM->SBUF bounce (512 cycles) narrows to bf16; 2/3 of copies run
  on ACT immediately, 1/3 on DVE deferred 2 steps (so the in-order DVE
  queue never waits on the ACT->PE round-trip), balancing ACT and DVE
  at ~82us each.

x loads are staged [1,3,4,...,4,2,1,1] timesteps (fast fill + short
drain) on the sync HWDGE ring; packed y is stored in [4,...,4,2,1,1]-
step batches on the scalar ring (DRAM layout [P, T, 512] so each
partition's chunk is one contiguous descriptor).

The kernel is DMA-bound at the chip HBM roofline: 33.55MB x in +
4.19MB y out per core at ~25.5GB/s/engine x 16 engines -> ~92us of DMA
engine time; DVE ~83us, ACT ~82us, PE ~22us hide under it. Measured
114.7-137us (median ~128) — run-to-run spread is cross-core HBM
contention (8 cores demand ~3.4TB/s burst vs ~2.9TB/s chip) plus an
occasional ~0.84x DVFS mode; engine times are otherwise stable.
"""

import sys

if "/opt/trn_rl_repo" not in sys.path:
    sys.path.insert(0, "/opt/trn_rl_repo")

import numpy as np

TAU = 0.5
V_TH = 1.0
C_TH = 1.0 - 2.0 ** -24      # largest f32 < V_TH

N_CORES = 8
T, B, N = 32, 64, 32768
B_SH = B // N_CORES          # 8 batch rows per core
E = B_SH * N                 # 262144 elements per core per timestep
P = 128                      # SBUF partitions
F = E // P                   # 2048 f32 per partition per timestep

PACK_BIAS = (4 ** 4 - 1) // 3  # 85: balanced -> unsigned base-4 digits
MM = 512                     # matmul moving free-dim limit; 4 bands per step

X_CHUNKS = [1, 3] + [4] * 6 + [2, 1, 1]  # timesteps per x load
Y_CHUNKS = [4] * 7 + [2, 1, 1]           # timesteps per packed-y store

_compiled = None


def _register_lif_op():
    """Register the fused LIF step as a custom DVE op (dve_ops.OPS append,
    the documented extension path; the uop table is generated per-NEFF)."""
    import concourse.dve_ops as dve_ops
    from concourse.dve_spec import (
        Spec, Src0, Src1, C0, C1, Zero, select, lower, _has_src1,
    )
    from concourse.dve_uop import DveOpSpec

    name = "LIF_STEP_ANT"
    for op in dve_ops.OPS:
        if op.name == name:
            return op
    body = select(Src0 < C1, Src0, Zero) * C0 + Src1
    spec = Spec(
        body=body,
        reference=lambda in0, in1, s0, s1, imm2: (
            np.where(in0 < s1, in0, np.float32(0.0)).astype(np.float32)
            * np.float32(s0) + in1
        ).astype(np.float32),
    )
    row = dve_ops._CUSTOM_DVE_ROW_BASE + len(dve_ops.OPS)
    assert row < 0x20
    dve_ops._SUB_OPCODE_FOR_NAME[name] = row
    sha = DveOpSpec(
        name=name, uops=lower(spec, ver="v3"), rd1_en=_has_src1(spec)
    ).sha("v3")
    op = dve_ops.DveOp(name, spec, subdim=False, uops_sha={"v3": sha})
    dve_ops.OPS.append(op)
    dve_ops.CUSTOM_DVE_SPECS[name] = spec
    return op


def _pack_weights() -> np.ndarray:
    """[128, 32] bf16: W[p, g] = 4^(p%4) for g == p//4 else 0 (exact in bf16)."""
    import ml_dtypes

    w = np.zeros((P, 32), dtype=np.float32)
    for p in range(P):
        w[p, p // 4] = float(4 ** (p % 4))
    return w.astype(ml_dtypes.bfloat16)


def _build():
    from concourse import bacc, tile, mybir
    from concourse.bass import MemorySpace

    lif_op = _register_lif_op()
    f32 = mybir.dt.float32
    bf16 = mybir.dt.bfloat16
    assert sum(X_CHUNKS) == T
    nc = bacc.Bacc("TRN2", debug=False, num_devices=N_CORES)
    x = nc.dram_tensor("x", [T, E], f32, kind="ExternalInput").ap()
    w = nc.dram_tensor("w", [P, 32], bf16, kind="ExternalInput").ap()
    yp = nc.dram_tensor("yp", [P, T, MM], bf16, kind="ExternalOutput").ap()

    x_r = x.rearrange("t (p f) -> t p f", p=P)  # [t, p, f] view of DRAM

    with tile.TileContext(nc) as tc:
        with (
            tc.tile_pool(name="io", bufs=3) as io_pool,
            tc.tile_pool(name="state", bufs=1) as st_pool,
            tc.tile_pool(name="m", bufs=6) as m_pool,
            tc.tile_pool(name="s", bufs=4) as s_pool,
            tc.tile_pool(name="ps", bufs=4, space=MemorySpace.PSUM) as ps_pool,
            tc.tile_pool(name="yb", bufs=3) as y_pool,
        ):
            cb = st_pool.tile([P, 1], f32, tag="cb")     # Sign bias = -C_TH
            nc.gpsimd.memset(cb[:], -C_TH)
            m_prev = st_pool.tile([P, F], f32, tag="m0")  # v[-1] = 0 seed
            nc.gpsimd.memset(m_prev[:], 0.0)
            wt = st_pool.tile([P, 32], bf16, tag="w")
            nc.sync.dma_start(out=wt[:], in_=w)

            # issue x loads lazily, two chunks ahead of consumption
            x_tiles = {}          # t -> (tile, col offset)
            next_chunk = 0
            t_loaded = 0

            def load_chunk():
                nonlocal next_chunk, t_loaded
                n_t = X_CHUNKS[next_chunk]
                xt = io_pool.tile([P, 4 * F], f32, tag="x")
                nc.sync.dma_start(
                    out=xt[:, : n_t * F].rearrange("p (t f) -> p t f", t=n_t),
                    in_=x_r[t_loaded:t_loaded + n_t].rearrange("t p f -> p t f"),
                )
                for i in range(n_t):
                    x_tiles[t_loaded + i] = (xt, i * F)
                next_chunk += 1
                t_loaded += n_t

            load_chunk()
            y_t = None
            y_chunk_idx = 0
            y_off = 0  # timesteps into current y chunk
            deferred = {}  # emit-step -> (psum tile, y slot) for DVE copies

            def flush_deferred(t_emit):
                # DVE PSUM->SBUF copy, deferred 2 steps so its matmul dep is
                # long done and the in-order DVE queue never stalls on it
                pk_d, yslot_d = deferred.pop(t_emit)
                nc.vector.tensor_scalar(
                    out=yslot_d, in0=pk_d[:], scalar1=1.0, scalar2=None,
                    op0=mybir.AluOpType.mult,
                )

            for t in range(T):
                if t not in x_tiles:
                    load_chunk()
                if next_chunk < len(X_CHUNKS) and t == t_loaded - X_CHUNKS[next_chunk - 1]:
                    load_chunk()  # prefetch one chunk ahead
                xt, off = x_tiles.pop(t)
                xs = xt[:, off:off + F]
                m = m_pool.tile([P, F], f32, tag="m")
                # m = select(m_prev < vth, m_prev, 0) * tau + x_t
                nc.vector._custom_dve(
                    lif_op, out=m[:], in0=m_prev[:], in1=xs, s0=TAU, s1=V_TH,
                )
                if t in deferred:
                    flush_deferred(t)
                # s = Sign(m - c) -> bf16 in {-1, 0, +1}; +1 iff m >= vth
                s = s_pool.tile([P, F], bf16, tag="s")
                nc.scalar.activation(
                    out=s[:], in_=m[:],
                    func=mybir.ActivationFunctionType.Sign,
                    bias=cb[:], scale=1.0,
                )
                m_prev = m
                # PE pack (4 signs -> one bf16-exact integer in [-85, 85]):
                # matmul q packs columns [512q, 512q+512) into the 32-row
                # band at PSUM base partition 32q — all 128 partitions hold
                # real data:  pk[32q+g, f'] = sum_{j<4} 4^j * s[4g+j, 512q+f']
                pk = ps_pool.tile([P, MM], f32, tag="pk")
                for q in range(4):
                    nc.tensor.matmul(
                        out=pk[32 * q:32 * (q + 1), :],
                        lhsT=wt[:],
                        rhs=s[:, MM * q:MM * (q + 1)],
                        start=True, stop=True,
                        tile_position=(0, 32 * q),
                    )
                # PSUM -> SBUF bounce (512 cycles) narrowing to bf16 (exact:
                # small integers). 2/3 on ACT immediately; 1/3 on DVE deferred
                # 2 steps, balancing ACT (~Sign+2/3 copy) against DVE.
                n_yt = Y_CHUNKS[y_chunk_idx]
                if y_off == 0:
                    y_t = y_pool.tile([P, max(Y_CHUNKS) * MM], bf16, tag="y")
                yslot = y_t[:, y_off * MM:(y_off + 1) * MM]
                if y_off < 2 and n_yt == 4:
                    # lands at y_off+2 in the same chunk, before its store
                    deferred[t + 2] = (pk, yslot)
                else:
                    nc.scalar.copy(out=yslot, in_=pk[:])
                y_off += 1
                if y_off == n_yt:
                    assert not deferred, deferred
                    nc.scalar.dma_start(
                        out=yp[:, t - n_yt + 1:t + 1, :],
                        in_=y_t[:, : n_yt * MM].rearrange("p (t f) -> p t f", t=n_yt),
                    )
                    y_chunk_idx += 1
                    y_off = 0
    nc.compile()
    return nc


def _get_compiled():
    global _compiled
    if _compiled is None:
        _compiled = _build()
        # warm the NEFF (first execution pays ~20us of cold-start)
        import concourse.bass_utils as bass_utils

        z = [
            {"x": np.zeros((T, E), dtype=np.float32), "w": _pack_weights()}
            for _ in range(N_CORES)
        ]
        bass_utils.run_bass_kernel_spmd(
            _compiled, z, core_ids=list(range(N_CORES))
        )
    return _compiled


def kernel(x: np.ndarray, _trace: bool = False):
    import concourse.bass_utils as bass_utils

    nc = _get_compiled()
    x = np.ascontiguousarray(x, dtype=np.float32)
    wv = _pack_weights()
    in_maps = [
        {"x": x[:, c * B_SH:(c + 1) * B_SH, :].reshape(T, E), "w": wv}
        for c in range(N_CORES)
    ]
    res = bass_utils.run_bass_kernel_spmd(
        nc, in_maps, core_ids=list(range(N_CORES)), trace=_trace
    )
    y = np.empty((T, B, N), dtype=np.float32)
    shifts = (2 * np.arange(4, dtype=np.int32))[None, None, :, None, None]
    for c in range(N_CORES):
        v32 = res.results[c]["yp"].astype(np.int32) + PACK_BIAS  # [128, T, 512]
        # row 32q + g, step t, word f', digit j -> element (4g+j, 512q+f')
        d = (v32.reshape(4, 32, 1, T, MM) >> shifts) & 3         # [q, g, j, t, f']
        yb = (d == 2).transpose(3, 1, 2, 0, 4)                   # [t, g, j, q, f']
        y[:, c * B_SH:(c + 1) * B_SH, :] = yb.reshape(T, B_SH, N)
    if _trace:
        return y, res
    return y


# revision 31
# speedup vs baseline: 1.1436x; 1.0118x over previous
"""LIF (leaky integrate-and-fire) recurrence kernel for Trainium2, 8 NeuronCores.

Problem: x (T=32, B=64, N=32768) f32.
    m[t] = tau*v[t-1] + x[t];  y[t] = (m[t] >= 1.0);  v[t] = m[t]*(1-y[t])
Output: y (32, 64, 32768) f32.

Sharding: data-parallel over batch. Core c handles x[:, 8c:8(c+1), :],
a (32, 262144)-element independent recurrence.

Per-core pipeline (bit-exact vs the f32 reference):
  DVE — ONE custom-DVE op per timestep (registered at import, the
  supported dve_ops.OPS extension path). State is m (not v); the reset
  folds into the next step's read:
      m[t] = select(m[t-1] < vth, m[t-1], 0) * tau + x[t]
  4 ALU stages, 1 elem/cycle -> ~2.2us/step instead of the 2-op
  scalar_tensor_tensor chain (~4.6us/step).
  ACT — s = Sign(m - c) with c = 1 - 2^-24 (the f32 just below vth):
  s = +1  <=>  m >= vth exactly (Sterbenz); s in {-1,0,+1} written bf16.
  PE  — packs 4 signs into one word: pk[32q+g, f'] = sum_{j<4} 4^j *
  s[4g+j, 512q+f'], a balanced base-4 integer in [-85, 85] — exact in
  bf16 (weights 4^j are exact powers of two; every product and partial
  sum is an exact small integer; matmul q fills the 32-partition band
  at PSUM base 32q, so all 128 partitions carry real data and the
  stores get full-rate 4KB descriptors). The host adds 85 and reads
  2-bit digits: y = (digit == 2). 0.5 B/element of y DMA traffic.
  The PSUM->SBUF bounce (512 cycles) narrows to bf16; 2/3 of copies run
  on ACT immediately, 1/3 on DVE deferred 2 steps (so the in-order DVE
  queue never waits on the ACT->PE round-trip), balancing ACT and DVE
  at ~82us each.

x loads are staged [1,3,4,...,4,2,1,1] timesteps (fast fill + short
drain) on the sync HWDGE ring; packed y is stored in [4,...,4,2,1,1]-
step batches on the scalar ring (DRAM layout [P, T, 512] so each
partition's chunk is one contiguous descriptor).

The kernel is DMA-bound at the chip HBM roofline: 33.55MB x in +
4.19MB y out per core at ~25.5GB/s/engine x 16 engines -> ~92us of DMA
engine time; DVE ~83us, ACT ~82us, PE ~22us hide under it. Measured
114.7-137us (median ~128) — run-to-run spread is cross-core HBM
contention (8 cores demand ~3.4TB/s burst vs ~2.9TB/s chip) plus an
occasional ~0.84x DVFS mode; engine times are otherwise stable.
"""

import sys

if "/opt/trn_rl_repo" not in sys.path:
    sys.path.insert(0, "/opt/trn_rl_repo")

import numpy as np

TAU = 0.5
V_TH = 1.0
C_TH = 1.0 - 2.0 ** -24      # largest f32 < V_TH

N_CORES = 8
T, B, N = 32, 64, 32768
B_SH = B // N_CORES          # 8 batch rows per core
E = B_SH * N                 # 262144 elements per core per timestep
P = 128                      # SBUF partitions
F = E // P                   # 2048 f32 per partition per timestep

PACK_BIAS = (4 ** 4 - 1) // 3  # 85: balanced -> unsigned base-4 digits
MM = 512                     # matmul moving free-dim limit; 4 bands per step

X_CHUNKS = [1, 3] + [4] * 6 + [2, 1, 1]  # timesteps per x load
Y_CHUNKS = [4] * 7 + [2, 1, 1]           # timesteps per packed-y store

_compiled = None


def _register_lif_op():
    """Register the fused LIF step as a custom DVE op (dve_ops.OPS append,
    the documented extension path; the uop table is generated per-NEFF)."""
    import concourse.dve_ops as dve_ops
    from concourse.dve_spec import (
        Spec, Src0, Src1, C0, C1, Zero, select, lower, _has_src1,
    )
    from concourse.dve_uop import DveOpSpec

    name = "LIF_STEP_ANT"
    for op in dve_ops.OPS:
        if op.name == name:
            return op
    body = select(Src0 < C1, Src0, Zero) * C0 + Src1
    spec = Spec(
        body=body,
        reference=lambda in0, in1, s0, s1, imm2: (
            np.where(in0 < s1, in0, np.float32(0.0)).astype(np.float32)
            * np.float32(s0) + in1
        ).astype(np.float32),
    )
    row = dve_ops._CUSTOM_DVE_ROW_BASE + len(dve_ops.OPS)
    assert row < 0x20
    dve_ops._SUB_OPCODE_FOR_NAME[name] = row
    sha = DveOpSpec(
        name=name, uops=lower(spec, ver="v3"), rd1_en=_has_src1(spec)
    ).sha("v3")
    op = dve_ops.DveOp(name, spec, subdim=False, uops_sha={"v3": sha})
    dve_ops.OPS.append(op)
    dve_ops.CUSTOM_DVE_SPECS[name] = spec
    return op


def _pack_weights() -> np.ndarray:
    """[128, 32] bf16: W[p, g] = 4^(p%4) for g == p//4 else 0 (exact in bf16)."""
    import ml_dtypes

    w = np.zeros((P, 32), dtype=np.float32)
    for p in range(P):
        w[p, p // 4] = float(4 ** (p % 4))
    return w.astype(ml_dtypes.bfloat16)


def _build():
    from concourse import bacc, tile, mybir
    from concourse.bass import MemorySpace

    lif_op = _register_lif_op()
    f32 = mybir.dt.float32
    bf16 = mybir.dt.bfloat16
    assert sum(X_CHUNKS) == T
    nc = bacc.Bacc("TRN2", debug=False, num_devices=N_CORES)
    x = nc.dram_tensor("x", [T, E], f32, kind="ExternalInput").ap()
    w = nc.dram_tensor("w", [P, 32], bf16, kind="ExternalInput").ap()
    yp = nc.dram_tensor("yp", [P, T, MM], bf16, kind="ExternalOutput").ap()

    x_r = x.rearrange("t (p f) -> t p f", p=P)  # [t, p, f] view of DRAM

    with tile.TileContext(nc) as tc:
        with (
            tc.tile_pool(name="io", bufs=3) as io_pool,
            tc.tile_pool(name="state", bufs=1) as st_pool,
            tc.tile_pool(name="m", bufs=6) as m_pool,
            tc.tile_pool(name="s", bufs=4) as s_pool,
            tc.tile_pool(name="ps", bufs=4, space=MemorySpace.PSUM) as ps_pool,
            tc.tile_pool(name="yb", bufs=3) as y_pool,
        ):
            cb = st_pool.tile([P, 1], f32, tag="cb")     # Sign bias = -C_TH
            nc.gpsimd.memset(cb[:], -C_TH)
            m_prev = st_pool.tile([P, F], f32, tag="m0")  # v[-1] = 0 seed
            nc.gpsimd.memset(m_prev[:], 0.0)
            wt = st_pool.tile([P, 32], bf16, tag="w")
            nc.sync.dma_start(out=wt[:], in_=w)

            # issue x loads lazily, two chunks ahead of consumption
            x_tiles = {}          # t -> (tile, col offset)
            next_chunk = 0
            t_loaded = 0

            def load_chunk():
                nonlocal next_chunk, t_loaded
                n_t = X_CHUNKS[next_chunk]
                xt = io_pool.tile([P, 4 * F], f32, tag="x")
                nc.sync.dma_start(
                    out=xt[:, : n_t * F].rearrange("p (t f) -> p t f", t=n_t),
                    in_=x_r[t_loaded:t_loaded + n_t].rearrange("t p f -> p t f"),
                )
                for i in range(n_t):
                    x_tiles[t_loaded + i] = (xt, i * F)
                next_chunk += 1
                t_loaded += n_t

            load_chunk()
            y_t = None
            y_chunk_idx = 0
            y_off = 0  # timesteps into current y chunk
            deferred = {}  # emit-step -> (psum tile, y slot) for DVE copies

            def flush_deferred(t_emit):
                # DVE PSUM->SBUF copy, deferred 2 steps so its matmul dep is
                # long done and the in-order DVE queue never stalls on it
                pk_d, yslot_d = deferred.pop(t_emit)
                nc.vector.tensor_scalar(
                    out=yslot_d, in0=pk_d[:], scalar1=1.0, scalar2=None,
                    op0=mybir.AluOpType.mult,
                )

            for t in range(T):
                if t not in x_tiles:
                    load_chunk()
                if next_chunk < len(X_CHUNKS) and t == t_loaded - X_CHUNKS[next_chunk - 1]:
                    load_chunk()  # prefetch one chunk ahead
                xt, off = x_tiles.pop(t)
                xs = xt[:, off:off + F]
                m = m_pool.tile([P, F], f32, tag="m")
                # m = select(m_prev < vth, m_prev, 0) * tau + x_t
                nc.vector._custom_dve(
                    lif_op, out=m[:], in0=m_prev[:], in1=xs, s0=TAU, s1=V_TH,
                )
                if t in deferred:
                    flush_deferred(t)
                # s = Sign(m - c) -> bf16 in {-1, 0, +1}; +1 iff m >= vth
                s = s_pool.tile([P, F], bf16, tag="s")
                nc.scalar.activation(
                    out=s[:], in_=m[:],
                    func=mybir.ActivationFunctionType.Sign,
                    bias=cb[:], scale=1.0,
                )
                m_prev = m
                # PE pack (4 signs -> one bf16-exact integer in [-85, 85]):
                # matmul q packs columns [512q, 512q+512) into the 32-row
                # band at PSUM base partition 32q — all 128 partitions hold
                # real data:  pk[32q+g, f'] = sum_{j<4} 4^j * s[4g+j, 512q+f']
                pk = ps_pool.tile([P, MM], f32, tag="pk")
                for q in range(4):
                    nc.tensor.matmul(
                        out=pk[32 * q:32 * (q + 1), :],
                        lhsT=wt[:],
                        rhs=s[:, MM * q:MM * (q + 1)],
                        start=True, stop=True,
                        tile_position=(0, 32 * q),
                    )
                # PSUM -> SBUF bounce (512 cycles) narrowing to bf16 (exact:
                # small integers). 2/3 on ACT immediately; 1/3 on DVE deferred
                # 2 steps, balancing ACT (~Sign+2/3 copy) against DVE.
                n_yt = Y_CHUNKS[y_chunk_idx]
                if y_off == 0:
                    y_t = y_pool.tile([P, max(Y_CHUNKS) * MM], bf16, tag="y")
                yslot = y_t[:, y_off * MM:(y_off + 1) * MM]
                if y_off < 2 and n_yt == 4:
                    # lands at y_off+2 in the same chunk, before its store
                    deferred[t + 2] = (pk, yslot)
                else:
                    nc.scalar.copy(out=yslot, in_=pk[:])
                y_off += 1
                if y_off == n_yt:
                    assert not deferred, deferred
                    nc.gpsimd.dma_start(
                        out=yp[:, t - n_yt + 1:t + 1, :],
                        in_=y_t[:, : n_yt * MM].rearrange("p (t f) -> p t f", t=n_yt),
                    )
                    y_chunk_idx += 1
                    y_off = 0
    nc.compile()
    return nc


def _get_compiled():
    global _compiled
    if _compiled is None:
        _compiled = _build()
        # warm the NEFF (first execution pays ~20us of cold-start)
        import concourse.bass_utils as bass_utils

        z = [
            {"x": np.zeros((T, E), dtype=np.float32), "w": _pack_weights()}
            for _ in range(N_CORES)
        ]
        bass_utils.run_bass_kernel_spmd(
            _compiled, z, core_ids=list(range(N_CORES))
        )
    return _compiled


def kernel(x: np.ndarray, _trace: bool = False):
    import concourse.bass_utils as bass_utils

    nc = _get_compiled()
    x = np.ascontiguousarray(x, dtype=np.float32)
    wv = _pack_weights()
    in_maps = [
        {"x": x[:, c * B_SH:(c + 1) * B_SH, :].reshape(T, E), "w": wv}
        for c in range(N_CORES)
    ]
    res = bass_utils.run_bass_kernel_spmd(
        nc, in_maps, core_ids=list(range(N_CORES)), trace=_trace
    )
    y = np.empty((T, B, N), dtype=np.float32)
    shifts = (2 * np.arange(4, dtype=np.int32))[None, None, :, None, None]
    for c in range(N_CORES):
        v32 = res.results[c]["yp"].astype(np.int32) + PACK_BIAS  # [128, T, 512]
        # row 32q + g, step t, word f', digit j -> element (4g+j, 512q+f')
        d = (v32.reshape(4, 32, 1, T, MM) >> shifts) & 3         # [q, g, j, t, f']
        yb = (d == 2).transpose(3, 1, 2, 0, 4)                   # [t, g, j, q, f']
        y[:, c * B_SH:(c + 1) * B_SH, :] = yb.reshape(T, B_SH, N)
    if _trace:
        return y, res
    return y
